# revision 38
# baseline (speedup 1.0000x reference)
"""Grouped SwiGLU MoE (16 experts, top-2, 8192x1024 tokens, d_ff 2816) on 8 TRN2 cores.

Expert-parallel, 2 experts per core. Host does the integer routing (sort
tokens by expert), pairs heavy experts with light ones so the two per-core
segment lengths (L0 >= L1) are data-tight instead of a global worst-case cap,
and pre-casts activations/weights to bf16 (PE runs bf16 at full rate; rel-err
budget 2e-2 >> bf16 noise ~3e-3). The routing weight is folded into a second,
pre-scaled copy of x that feeds the w2 (value) path - the w3 GEMM is linear,
so the final per-token multiply disappears and the output can stream to HBM
during the last weight group. All tensors are pre-arranged on the host into
the exact SBUF [partition, free] layouts so every DMA is 128 long contiguous
rows (DMA-descriptor efficiency ~ the per-ring bandwidth limiter). Input
loads ride the SP HWDGE queue (the sync engine has no compute duties, so
ring-full waits on issue instructions are free); output stores ride the Act
queue. A small first weight group (2 f-tiles) plus smallest-slice-first
ordering lets the PE start ~12us into the kernel. Everything device-side is
Bass/Tile via run_bass_kernel_spmd on cores 0-7.

Per-core compute: y^T = w3^T @ (silu(w1^T x^T) * (w2^T (w.x)^T)), all in
feature-major layout so no on-chip transposes are needed.
"""

import numpy as np

N_EXPERTS, D_MODEL, D_FF = 16, 1024, 2816
N_TOKENS, TOP_K = 8192, 2
N_CORES = 8
E_LOCAL = N_EXPERTS // N_CORES  # 2 expert slots per core
DK = D_MODEL // 128             # 8 contraction tiles for x
FK = D_FF // 128                # 22 f tiles
# f-tiles per streamed weight group; a small first group lets the PE start
# after only ~0.75MB of weights has landed
GROUPS = [2, 4, 4, 4, 4, 4]
assert sum(GROUPS) == FK
NG = len(GROUPS)
F_GROUP = max(GROUPS)
GOFF = [sum(GROUPS[:i]) for i in range(NG)]  # f-tile offset per group

# ----------------------------------------------------------------- host utils


def _to_bf16(x: np.ndarray) -> np.ndarray:
    import ml_dtypes

    return np.ascontiguousarray(x, dtype=np.float32).astype(ml_dtypes.bfloat16)


def _slice_plan(L: int) -> list[tuple[int, int]]:
    """Split L into (start, width) pieces, all in [256, 512] where possible
    (the PE runs at half rate below 256 free elements)."""
    out, s = [], 0
    rem = L
    while rem > 512:
        w = 512 if rem >= 768 else rem - 256
        out.append((s, w))
        s += w
        rem -= w
    out.append((s, rem))
    return out


# ------------------------------------------------------- walrus wait-split fix


def _split_excess_waits(nc):
    """This walrus build encodes at most ONE sync wait per instruction; Tile
    can attach several (first matmul of a group, kernel-tail drain). Hoist the
    excess into standalone InstEventSemaphore (the shape wait_ge emits)."""
    import bass_rust
    import concourse.mybir as mybir

    n = 0
    for fn in nc.m.functions:
        for blk in fn.blocks:
            out, changed = [], False
            for inst in blk.instructions:
                si = inst.sync_info
                if si is not None and si.on_wait is not None and len(si.on_wait) > 1:
                    waits = list(si.on_wait)
                    for w in waits[:-1]:
                        ev = mybir.InstEventSemaphore(name=f"I-wsplit-{n}", ins=[], outs=[])
                        n += 1
                        ev.engine = inst.engine
                        ev.sync_info = bass_rust.SyncInfo(on_wait=[w], on_update=[])
                        out.append(ev)
                    inst.sync_info = bass_rust.SyncInfo(
                        on_wait=waits[-1:], on_update=list(si.on_update or [])
                    )
                    changed = True
                out.append(inst)
            if changed:
                blk.instructions = out
    return n


# ------------------------------------------------------------- device program


def _build(L0: int, L1: int):
    import concourse.bass as bass
    import concourse.tile as tile
    import concourse.mybir as mybir

    f32 = mybir.dt.float32
    bf16 = mybir.dt.bfloat16
    Ls = (L0, L1)
    plans = [_slice_plan(L) for L in Ls]

    nc = bass.Bass()
    # x arrives pre-transposed to [128, 2, DK, w] per token-slice: plane 0 is
    # x (gate path), plane 1 is x pre-scaled by the routing weight (value
    # path). One DMA per slice, 16KB rows.
    xa_d = [[nc.dram_tensor(f"xa{e}s{k}", [128, 2, DK, w], bf16, kind="ExternalInput")
             for k, (s0, w) in enumerate(plans[e])] for e in range(E_LOCAL)]
    # weights pre-arranged group-major, flat (no padding): per group the
    # layout is [p][2][dk][f] for w1+w2 stacked, [p][fl][dk][d] for w3
    WTOT = 128 * DK * FK  # w3 free elements per partition row
    w12t = nc.dram_tensor("w12t", [E_LOCAL, 128, 2 * WTOT], bf16, kind="ExternalInput")
    w3t = nc.dram_tensor("w3t", [E_LOCAL, 128, WTOT], bf16, kind="ExternalInput")
    yt_d = [[nc.dram_tensor(f"yt{e}s{k}", [128, DK, w], bf16, kind="ExternalOutput")
             for k, (s0, w) in enumerate(plans[e])] for e in range(E_LOCAL)]

    with tile.TileContext(nc) as tc:
        with (
            tc.tile_pool(name="xts", bufs=1) as p_x,
            tc.tile_pool(name="w12", bufs=2) as p_w12,
            tc.tile_pool(name="w3", bufs=2) as p_w3,
            tc.tile_pool(name="hs", bufs=2 * F_GROUP) as p_hs,
            tc.tile_pool(name="sil", bufs=3) as p_sil,
            tc.tile_pool(name="yacc", bufs=1) as p_y,
            tc.tile_pool(name="yo", bufs=4) as p_yo,
            tc.tile_pool(name="gv", bufs=4, space="PSUM") as p_gv,
            tc.tile_pool(name="py", bufs=2, space="PSUM") as p_py,
        ):
            # warm the scalar engine's SILU table before real work arrives so
            # ACT_TABLE_LOAD (~1.3us) is off the critical path
            warm = p_sil.tile([128, 8], f32, tag="silwarm")
            nc.vector.memset(warm[:], 0.0)
            nc.scalar.activation(warm[:], warm[:], mybir.ActivationFunctionType.Silu)

            # stage all x up-front on the SP queue (the sync engine has no
            # compute duties, so ring-full waits on DMA-issue instructions
            # cost nothing), one contiguous [128, 2*DK*w] slab per
            # token-slice. Smallest slice first: group 0 computes it first,
            # so the PE can start after a minimal transfer.
            xa_t = []
            for e in range(E_LOCAL):
                xa_t.append([p_x.tile([128, 2, DK, w], bf16, tag=f"xa{e}s{k}",
                                      name=f"xa{e}s{k}")
                             for k, (s0, w) in enumerate(plans[e])])

            def slice_order(e, first):
                idx = list(range(len(plans[e])))
                if first:
                    idx.sort(key=lambda k: plans[e][k][1])
                return idx

            # x-load issue schedule, interleaved with the weight-group loads
            # on the same SP queue: expert 0's slices (smallest first) come
            # right after group 0's w12; expert 1's trickle in during
            # expert 0's early groups.
            x_issue: dict[tuple[int, int], list[tuple[int, int]]] = {}
            x_issue[(0, 0)] = [(0, k) for k in slice_order(0, True)]
            e1_slices = [(1, k) for k in slice_order(1, True)] if E_LOCAL > 1 else []
            for i, item in enumerate(e1_slices):
                x_issue.setdefault((0, min(1 + i // 2, NG - 1)), []).append(item)

            for e in range(E_LOCAL):
                L = Ls[e]
                slices = plans[e]
                y_acc = p_y.tile([128, DK, Ls[0]], f32, tag="yacc")

                for gi in range(NG):
                    glen = GROUPS[gi]
                    last_g = gi == NG - 1
                    woff = GOFF[gi] * DK * 128 * 2
                    wlen = glen * DK * 128 * 2
                    w3off = GOFF[gi] * DK * 128
                    w3len = glen * DK * 128
                    w12r = p_w12.tile([128, 2, DK, glen * 128], bf16, tag="w12r")
                    w3r = p_w3.tile([128, glen, DK, 128], bf16, tag="w3r")
                    xq = list(x_issue.get((e, gi), []))
                    if gi == 0 and e == 0 and xq:
                        # startup critical path: split the first x slab and
                        # the w12 group across rings so each lands in ~2us
                        xe, xk = xq.pop(0)
                        hw = wlen // 2
                        nc.sync.dma_start(
                            out=w12r[:, 0], in_=w12t[e, :, woff:woff + hw]
                        )
                        nc.sync.dma_start(
                            out=xa_t[xe][xk][:, 0, :DK // 2],
                            in_=xa_d[xe][xk][:, 0, :DK // 2],
                        )
                        nc.sync.dma_start(
                            out=xa_t[xe][xk][:, 0, DK // 2:],
                            in_=xa_d[xe][xk][:, 0, DK // 2:],
                        )
                        nc.sync.dma_start(
                            out=w12r[:, 1], in_=w12t[e, :, woff + hw:woff + wlen]
                        )
                        nc.sync.dma_start(out=xa_t[xe][xk][:, 1], in_=xa_d[xe][xk][:, 1])
                    else:
                        nc.sync.dma_start(out=w12r[:], in_=w12t[e, :, woff:woff + wlen])
                    nc.sync.dma_start(out=w3r[:], in_=w3t[e, :, w3off:w3off + w3len])
                    for xe, xk in xq:
                        nc.sync.dma_start(out=xa_t[xe][xk][:], in_=xa_d[xe][xk][:])

                    for k in slice_order(e, gi == 0):
                        s0, w = plans[e][k]
                        xa = xa_t[e][k]
                        hs_tiles = []
                        for fl in range(glen):
                            pg = p_gv.tile([128, 512], f32, tag="gv")
                            for dk in range(DK):
                                nc.tensor.matmul(
                                    pg[:, :w],
                                    w12r[:, 0, dk, fl * 128:(fl + 1) * 128],
                                    xa[:, 0, dk, :],
                                    start=(dk == 0),
                                    stop=(dk == DK - 1),
                                )
                            pv = p_gv.tile([128, 512], f32, tag="gv")
                            for dk in range(DK):
                                nc.tensor.matmul(
                                    pv[:, :w],
                                    w12r[:, 1, dk, fl * 128:(fl + 1) * 128],
                                    xa[:, 1, dk, :],
                                    start=(dk == 0),
                                    stop=(dk == DK - 1),
                                )
                            sil = p_sil.tile([128, 512], bf16, tag="sil")
                            nc.scalar.activation(
                                sil[:, :w], pg[:, :w], mybir.ActivationFunctionType.Silu
                            )
                            hst = p_hs.tile([128, 512], bf16, tag="hs")
                            nc.vector.tensor_mul(hst[:, :w], sil[:, :w], pv[:, :w])
                            hs_tiles.append(hst)

                        # the very last slice of the kernel streams per-di so
                        # its output DMA isn't serialized behind all 8 adds
                        final_unit = (
                            last_g and e == E_LOCAL - 1 and k == len(slices) - 1
                        )
                        yo = None
                        if last_g and not final_unit:
                            yo = p_yo.tile([128, DK, w], bf16, tag="yo")
                        for di in range(DK):
                            py = p_py.tile([128, 512], f32, tag="py")
                            for fl in range(glen):
                                nc.tensor.matmul(
                                    py[:, :w],
                                    w3r[:, fl, di, :],
                                    hs_tiles[fl][:, :w],
                                    start=(fl == 0),
                                    stop=(fl == glen - 1),
                                )
                            if gi == 0:
                                nc.vector.tensor_copy(y_acc[:, di, s0:s0 + w], py[:, :w])
                            elif final_unit:
                                yod = p_yo.tile([128, 512], bf16, tag="yod")
                                nc.vector.tensor_add(
                                    yod[:, :w], y_acc[:, di, s0:s0 + w], py[:, :w]
                                )
                                nc.scalar.dma_start(
                                    out=yt_d[e][k][:, di, :], in_=yod[:, :w]
                                )
                            elif last_g:
                                # final accumulation rounds to bf16 into a
                                # slice-wide slab that streams straight out
                                nc.vector.tensor_add(
                                    yo[:, di, :], y_acc[:, di, s0:s0 + w], py[:, :w]
                                )
                            else:
                                nc.vector.tensor_add(
                                    y_acc[:, di, s0:s0 + w],
                                    y_acc[:, di, s0:s0 + w],
                                    py[:, :w],
                                )
                        if last_g and not final_unit:
                            nc.scalar.dma_start(out=yt_d[e][k][:], in_=yo[:])

    _split_excess_waits(nc)
    return nc


_BUILD_CACHE: dict[tuple, object] = {}


def _get_nc(L0: int, L1: int):
    key = (L0, L1)
    if key not in _BUILD_CACHE:
        _BUILD_CACHE[key] = _build(L0, L1)
    return _BUILD_CACHE[key]


# -------------------------------------------------------------------- kernel


def _pack_w12(w1: np.ndarray, w2: np.ndarray) -> np.ndarray:
    """2x [D_MODEL, D_FF] f32 -> [128, 2*DK*FK*128] bf16, flat group-major:
    per group the per-partition layout is [2][dk][f_in_group]."""
    w1b, w2b = _to_bf16(w1), _to_bf16(w2)
    parts = []
    for gi, glen in enumerate(GROUPS):
        fb = GOFF[gi] * 128
        blk = np.stack(
            [wb[:, fb:fb + glen * 128].reshape(DK, 128, glen * 128)
             for wb in (w1b, w2b)]
        )                                                     # [2, DK, 128, gw]
        parts.append(blk.transpose(2, 0, 1, 3).reshape(128, -1))
    return np.ascontiguousarray(np.concatenate(parts, axis=1))


def _pack_w3(w: np.ndarray) -> np.ndarray:
    """[D_FF, D_MODEL] f32 -> [128, FK*DK*128] bf16, flat group-major:
    per group the per-partition layout is [fl][dk][d]."""
    wb = _to_bf16(w)
    parts = []
    for gi, glen in enumerate(GROUPS):
        fb = GOFF[gi] * 128
        blk = wb[fb:fb + glen * 128, :]                       # [gw, D_MODEL]
        parts.append(
            blk.reshape(glen, 128, DK, 128).transpose(1, 0, 2, 3).reshape(128, -1)
        )
    return np.ascontiguousarray(np.concatenate(parts, axis=1))


def prepare(x, expert_indices, expert_weights, w1, w2, w3):
    """Host routing + sharding. Returns (nc, in_maps, meta)."""
    x = np.asarray(x, dtype=np.float32)
    ei = np.asarray(expert_indices).reshape(-1)
    ew = np.asarray(expert_weights).reshape(-1).astype(np.float32)

    # ---- integer routing on host (replicated bookkeeping)
    order = np.argsort(ei, kind="stable")
    tok_sorted = (np.repeat(np.arange(N_TOKENS, dtype=np.int64), TOP_K))[order]
    w_sorted = ew[order]
    counts = np.bincount(ei, minlength=N_EXPERTS)
    seg = np.concatenate(([0], np.cumsum(counts)))

    # ---- pair heavy experts with light ones; slot 0 takes the bigger one
    by_size = np.argsort(counts, kind="stable")[::-1]  # expert ids, desc count
    slot_expert = np.zeros((N_CORES, E_LOCAL), dtype=np.int64)
    for c in range(N_CORES):
        slot_expert[c, 0] = by_size[c]
        slot_expert[c, 1] = by_size[N_EXPERTS - 1 - c]
    L0 = int(max(256, -(-int(counts[slot_expert[:, 0]].max()) // 4) * 4))
    L1 = int(max(256, -(-int(counts[slot_expert[:, 1]].max()) // 4) * 4))
    Ls = (L0, L1)
    plans = [_slice_plan(L) for L in Ls]

    xb = _to_bf16(x)
    import ml_dtypes

    # ---- shard per core
    WTOT = 128 * DK * FK
    in_maps = []
    for c in range(N_CORES):
        m = {}
        w12_c = np.empty((E_LOCAL, 128, 2 * WTOT), dtype=ml_dtypes.bfloat16)
        w3_c = np.empty((E_LOCAL, 128, WTOT), dtype=ml_dtypes.bfloat16)
        for e in range(E_LOCAL):
            g = int(slot_expert[c, e])
            L = Ls[e]
            toks = tok_sorted[seg[g]:seg[g + 1]]
            wts = w_sorted[seg[g]:seg[g + 1]]
            # [128, 2, DK, L]: xa[p, 0, dk, t] = x[tok_t, dk*128 + p],
            # plane 1 pre-scaled by the routing weight
            xa = np.zeros((128, 2, DK, L), dtype=ml_dtypes.bfloat16)
            xa[:, 0, :, :len(toks)] = (
                xb[toks].T.reshape(DK, 128, -1).transpose(1, 0, 2)
            )
            xa[:, 1, :, :len(toks)] = (
                _to_bf16(x[toks] * wts[:, None]).T.reshape(DK, 128, -1).transpose(1, 0, 2)
            )
            for k, (s0, w) in enumerate(plans[e]):
                m[f"xa{e}s{k}"] = np.ascontiguousarray(xa[:, :, :, s0:s0 + w])
            w12_c[e] = _pack_w12(np.asarray(w1)[g], np.asarray(w2)[g])
            w3_c[e] = _pack_w3(np.asarray(w3)[g])
        m["w12t"] = w12_c
        m["w3t"] = w3_c
        in_maps.append(m)

    nc = _get_nc(L0, L1)
    meta = {"seg": seg, "tok_sorted": tok_sorted, "Ls": Ls, "slot_expert": slot_expert}
    return nc, in_maps, meta


def combine(results, meta):
    """Unshard per-core expert outputs and sum the top-2 contributions."""
    seg, tok_sorted, Ls = meta["seg"], meta["tok_sorted"], meta["Ls"]
    slot_expert = meta["slot_expert"]
    assign_rows = np.empty((N_TOKENS * TOP_K, D_MODEL), dtype=np.float32)
    for c in range(N_CORES):
        for e in range(E_LOCAL):
            g = int(slot_expert[c, e])
            cnt = seg[g + 1] - seg[g]
            # yt{e}s{k} is [128, DK, w] with yt[p, dk, t] = y[d=dk*128+p][t]
            parts = [
                np.asarray(results[c][f"yt{e}s{k}"], dtype=np.float32)
                .transpose(1, 0, 2)
                .reshape(D_MODEL, -1)
                for k in range(len(_slice_plan(Ls[e])))
            ]
            ytc = np.concatenate(parts, axis=1)
            assign_rows[seg[g]:seg[g + 1]] = ytc[:, :cnt].T

    by_token = np.argsort(tok_sorted, kind="stable")
    out = assign_rows[by_token].reshape(N_TOKENS, TOP_K, D_MODEL).sum(axis=1)
    return out.astype(np.float32)


def kernel(x, expert_indices, expert_weights, w1, w2, w3, _run_opts=None):
    from concourse.bass_utils import run_bass_kernel_spmd

    nc, in_maps, meta = prepare(x, expert_indices, expert_weights, w1, w2, w3)
    opts = dict(_run_opts or {})
    res = run_bass_kernel_spmd(nc, in_maps, list(range(N_CORES)), **opts)
    if _run_opts is not None:
        _run_opts["result"] = res
    return combine(res.results, meta)


# revision 39
# speedup vs baseline: 1.0078x; 1.0078x over previous
"""Grouped SwiGLU MoE (16 experts, top-2, 8192x1024 tokens, d_ff 2816) on 8 TRN2 cores.

Expert-parallel, 2 experts per core. Host does the integer routing (sort
tokens by expert), pairs heavy experts with light ones so the two per-core
segment lengths (L0 >= L1) are data-tight instead of a global worst-case cap,
and pre-casts activations/weights to bf16 (PE runs bf16 at full rate; rel-err
budget 2e-2 >> bf16 noise ~3e-3). The routing weight is folded into a second,
pre-scaled copy of x that feeds the w2 (value) path - the w3 GEMM is linear,
so the final per-token multiply disappears and the output can stream to HBM
during the last weight group. All tensors are pre-arranged on the host into
the exact SBUF [partition, free] layouts so every DMA is 128 long contiguous
rows (DMA-descriptor efficiency ~ the per-ring bandwidth limiter). Input
loads ride the SP HWDGE queue (the sync engine has no compute duties, so
ring-full waits on issue instructions are free); output stores ride the Act
queue. A small first weight group (2 f-tiles) plus smallest-slice-first
ordering lets the PE start ~12us into the kernel. Everything device-side is
Bass/Tile via run_bass_kernel_spmd on cores 0-7.

Per-core compute: y^T = w3^T @ (silu(w1^T x^T) * (w2^T (w.x)^T)), all in
feature-major layout so no on-chip transposes are needed.
"""

import numpy as np

N_EXPERTS, D_MODEL, D_FF = 16, 1024, 2816
N_TOKENS, TOP_K = 8192, 2
N_CORES = 8
E_LOCAL = N_EXPERTS // N_CORES  # 2 expert slots per core
DK = D_MODEL // 128             # 8 contraction tiles for x
FK = D_FF // 128                # 22 f tiles
# f-tiles per streamed weight group; a small first group lets the PE start
# after only ~0.75MB of weights has landed
GROUPS = [2, 4, 4, 4, 4, 4]
assert sum(GROUPS) == FK
NG = len(GROUPS)
F_GROUP = max(GROUPS)
GOFF = [sum(GROUPS[:i]) for i in range(NG)]  # f-tile offset per group

# ----------------------------------------------------------------- host utils


def _to_bf16(x: np.ndarray) -> np.ndarray:
    import ml_dtypes

    return np.ascontiguousarray(x, dtype=np.float32).astype(ml_dtypes.bfloat16)


def _slice_plan(L: int) -> list[tuple[int, int]]:
    """Split L into (start, width) pieces, all in [256, 512] where possible
    (the PE runs at half rate below 256 free elements)."""
    out, s = [], 0
    rem = L
    while rem > 512:
        w = 512 if rem >= 768 else rem - 256
        out.append((s, w))
        s += w
        rem -= w
    out.append((s, rem))
    return out


# ------------------------------------------------------- walrus wait-split fix


def _split_excess_waits(nc):
    """This walrus build encodes at most ONE sync wait per instruction; Tile
    can attach several (first matmul of a group, kernel-tail drain). Hoist the
    excess into standalone InstEventSemaphore (the shape wait_ge emits)."""
    import bass_rust
    import concourse.mybir as mybir

    n = 0
    for fn in nc.m.functions:
        for blk in fn.blocks:
            out, changed = [], False
            for inst in blk.instructions:
                si = inst.sync_info
                if si is not None and si.on_wait is not None and len(si.on_wait) > 1:
                    waits = list(si.on_wait)
                    for w in waits[:-1]:
                        ev = mybir.InstEventSemaphore(name=f"I-wsplit-{n}", ins=[], outs=[])
                        n += 1
                        ev.engine = inst.engine
                        ev.sync_info = bass_rust.SyncInfo(on_wait=[w], on_update=[])
                        out.append(ev)
                    inst.sync_info = bass_rust.SyncInfo(
                        on_wait=waits[-1:], on_update=list(si.on_update or [])
                    )
                    changed = True
                out.append(inst)
            if changed:
                blk.instructions = out
    return n


# ------------------------------------------------------------- device program


def _build(L0: int, L1: int):
    import concourse.bass as bass
    import concourse.tile as tile
    import concourse.mybir as mybir

    f32 = mybir.dt.float32
    bf16 = mybir.dt.bfloat16
    Ls = (L0, L1)
    plans = [_slice_plan(L) for L in Ls]

    nc = bass.Bass()
    # x arrives pre-transposed to [128, 2, DK, w] per token-slice: plane 0 is
    # x (gate path), plane 1 is x pre-scaled by the routing weight (value
    # path). One DMA per slice, 16KB rows.
    xa_d = [[nc.dram_tensor(f"xa{e}s{k}", [128, 2, DK, w], bf16, kind="ExternalInput")
             for k, (s0, w) in enumerate(plans[e])] for e in range(E_LOCAL)]
    # weights pre-arranged group-major, flat (no padding): per group the
    # layout is [p][2][dk][f] for w1+w2 stacked, [p][fl][dk][d] for w3
    WTOT = 128 * DK * FK  # w3 free elements per partition row
    w12t = nc.dram_tensor("w12t", [E_LOCAL, 128, 2 * WTOT], bf16, kind="ExternalInput")
    w3t = nc.dram_tensor("w3t", [E_LOCAL, 128, WTOT], bf16, kind="ExternalInput")
    yt_d = [[nc.dram_tensor(f"yt{e}s{k}", [128, DK, w], bf16, kind="ExternalOutput")
             for k, (s0, w) in enumerate(plans[e])] for e in range(E_LOCAL)]

    with tile.TileContext(nc) as tc:
        with (
            tc.tile_pool(name="xts", bufs=1) as p_x,
            tc.tile_pool(name="w12", bufs=2) as p_w12,
            tc.tile_pool(name="w3", bufs=2) as p_w3,
            tc.tile_pool(name="hs", bufs=2 * F_GROUP) as p_hs,
            tc.tile_pool(name="sil", bufs=3) as p_sil,
            tc.tile_pool(name="yacc", bufs=1) as p_y,
            tc.tile_pool(name="yo", bufs=4) as p_yo,
            tc.tile_pool(name="gv", bufs=4, space="PSUM") as p_gv,
            tc.tile_pool(name="py", bufs=2, space="PSUM") as p_py,
        ):
            # warm the scalar engine's SILU table before real work arrives so
            # ACT_TABLE_LOAD (~1.3us) is off the critical path
            warm = p_sil.tile([128, 8], f32, tag="silwarm")
            nc.vector.memset(warm[:], 0.0)
            nc.scalar.activation(warm[:], warm[:], mybir.ActivationFunctionType.Silu)

            # stage all x up-front on the SP queue (the sync engine has no
            # compute duties, so ring-full waits on DMA-issue instructions
            # cost nothing), one contiguous [128, 2*DK*w] slab per
            # token-slice. Smallest slice first: group 0 computes it first,
            # so the PE can start after a minimal transfer.
            xa_t = []
            for e in range(E_LOCAL):
                xa_t.append([p_x.tile([128, 2, DK, w], bf16, tag=f"xa{e}s{k}",
                                      name=f"xa{e}s{k}")
                             for k, (s0, w) in enumerate(plans[e])])

            def slice_order(e, first):
                idx = list(range(len(plans[e])))
                if first:
                    idx.sort(key=lambda k: plans[e][k][1])
                return idx

            # x-load issue schedule, interleaved with the weight-group loads
            # on the same SP queue: expert 0's slices (smallest first) come
            # right after group 0's w12; expert 1's trickle in during
            # expert 0's early groups.
            x_issue: dict[tuple[int, int], list[tuple[int, int]]] = {}
            x_issue[(0, 0)] = [(0, k) for k in slice_order(0, True)]
            e1_slices = [(1, k) for k in slice_order(1, True)] if E_LOCAL > 1 else []
            for i, item in enumerate(e1_slices):
                x_issue.setdefault((0, min(1 + i // 2, NG - 1)), []).append(item)

            for e in range(E_LOCAL):
                L = Ls[e]
                slices = plans[e]
                y_acc = p_y.tile([128, DK, Ls[0]], f32, tag="yacc")

                for gi in range(NG):
                    glen = GROUPS[gi]
                    last_g = gi == NG - 1
                    woff = GOFF[gi] * DK * 128 * 2
                    wlen = glen * DK * 128 * 2
                    w3off = GOFF[gi] * DK * 128
                    w3len = glen * DK * 128
                    w12r = p_w12.tile([128, 2, DK, glen * 128], bf16, tag="w12r")
                    w3r = p_w3.tile([128, glen, DK, 128], bf16, tag="w3r")
                    xq = list(x_issue.get((e, gi), []))
                    nc.sync.dma_start(out=w12r[:], in_=w12t[e, :, woff:woff + wlen])
                    if xq and gi == 0 and e == 0:
                        # the first slab gates the very first GEMM: split its
                        # two planes across two rings so they land in parallel
                        xe, xk = xq.pop(0)
                        nc.sync.dma_start(out=xa_t[xe][xk][:, 0], in_=xa_d[xe][xk][:, 0])
                        nc.sync.dma_start(out=xa_t[xe][xk][:, 1], in_=xa_d[xe][xk][:, 1])
                    nc.sync.dma_start(out=w3r[:], in_=w3t[e, :, w3off:w3off + w3len])
                    for xe, xk in xq:
                        nc.sync.dma_start(out=xa_t[xe][xk][:], in_=xa_d[xe][xk][:])

                    for k in slice_order(e, gi == 0):
                        s0, w = plans[e][k]
                        xa = xa_t[e][k]
                        hs_tiles = []
                        for fl in range(glen):
                            pg = p_gv.tile([128, 512], f32, tag="gv")
                            for dk in range(DK):
                                nc.tensor.matmul(
                                    pg[:, :w],
                                    w12r[:, 0, dk, fl * 128:(fl + 1) * 128],
                                    xa[:, 0, dk, :],
                                    start=(dk == 0),
                                    stop=(dk == DK - 1),
                                )
                            pv = p_gv.tile([128, 512], f32, tag="gv")
                            for dk in range(DK):
                                nc.tensor.matmul(
                                    pv[:, :w],
                                    w12r[:, 1, dk, fl * 128:(fl + 1) * 128],
                                    xa[:, 1, dk, :],
                                    start=(dk == 0),
                                    stop=(dk == DK - 1),
                                )
                            sil = p_sil.tile([128, 512], bf16, tag="sil")
                            nc.scalar.activation(
                                sil[:, :w], pg[:, :w], mybir.ActivationFunctionType.Silu
                            )
                            hst = p_hs.tile([128, 512], bf16, tag="hs")
                            nc.vector.tensor_mul(hst[:, :w], sil[:, :w], pv[:, :w])
                            hs_tiles.append(hst)

                        # the very last slice of the kernel streams per-di so
                        # its output DMA isn't serialized behind all 8 adds
                        final_unit = (
                            last_g and e == E_LOCAL - 1 and k == len(slices) - 1
                        )
                        yo = None
                        if last_g and not final_unit:
                            yo = p_yo.tile([128, DK, w], bf16, tag="yo")
                        for di in range(DK):
                            py = p_py.tile([128, 512], f32, tag="py")
                            for fl in range(glen):
                                nc.tensor.matmul(
                                    py[:, :w],
                                    w3r[:, fl, di, :],
                                    hs_tiles[fl][:, :w],
                                    start=(fl == 0),
                                    stop=(fl == glen - 1),
                                )
                            if gi == 0:
                                nc.vector.tensor_copy(y_acc[:, di, s0:s0 + w], py[:, :w])
                            elif final_unit:
                                yod = p_yo.tile([128, 512], bf16, tag="yod")
                                nc.vector.tensor_add(
                                    yod[:, :w], y_acc[:, di, s0:s0 + w], py[:, :w]
                                )
                                nc.scalar.dma_start(
                                    out=yt_d[e][k][:, di, :], in_=yod[:, :w]
                                )
                            elif last_g:
                                # final accumulation rounds to bf16 into a
                                # slice-wide slab that streams straight out
                                nc.vector.tensor_add(
                                    yo[:, di, :], y_acc[:, di, s0:s0 + w], py[:, :w]
                                )
                            else:
                                nc.vector.tensor_add(
                                    y_acc[:, di, s0:s0 + w],
                                    y_acc[:, di, s0:s0 + w],
                                    py[:, :w],
                                )
                        if last_g and not final_unit:
                            nc.scalar.dma_start(out=yt_d[e][k][:], in_=yo[:])

    _split_excess_waits(nc)
    return nc


_BUILD_CACHE: dict[tuple, object] = {}


def _get_nc(L0: int, L1: int):
    key = (L0, L1)
    if key not in _BUILD_CACHE:
        _BUILD_CACHE[key] = _build(L0, L1)
    return _BUILD_CACHE[key]


# -------------------------------------------------------------------- kernel


def _pack_w12(w1: np.ndarray, w2: np.ndarray) -> np.ndarray:
    """2x [D_MODEL, D_FF] f32 -> [128, 2*DK*FK*128] bf16, flat group-major:
    per group the per-partition layout is [2][dk][f_in_group]."""
    w1b, w2b = _to_bf16(w1), _to_bf16(w2)
    parts = []
    for gi, glen in enumerate(GROUPS):
        fb = GOFF[gi] * 128
        blk = np.stack(
            [wb[:, fb:fb + glen * 128].reshape(DK, 128, glen * 128)
             for wb in (w1b, w2b)]
        )                                                     # [2, DK, 128, gw]
        parts.append(blk.transpose(2, 0, 1, 3).reshape(128, -1))
    return np.ascontiguousarray(np.concatenate(parts, axis=1))


def _pack_w3(w: np.ndarray) -> np.ndarray:
    """[D_FF, D_MODEL] f32 -> [128, FK*DK*128] bf16, flat group-major:
    per group the per-partition layout is [fl][dk][d]."""
    wb = _to_bf16(w)
    parts = []
    for gi, glen in enumerate(GROUPS):
        fb = GOFF[gi] * 128
        blk = wb[fb:fb + glen * 128, :]                       # [gw, D_MODEL]
        parts.append(
            blk.reshape(glen, 128, DK, 128).transpose(1, 0, 2, 3).reshape(128, -1)
        )
    return np.ascontiguousarray(np.concatenate(parts, axis=1))


def prepare(x, expert_indices, expert_weights, w1, w2, w3):
    """Host routing + sharding. Returns (nc, in_maps, meta)."""
    x = np.asarray(x, dtype=np.float32)
    ei = np.asarray(expert_indices).reshape(-1)
    ew = np.asarray(expert_weights).reshape(-1).astype(np.float32)

    # ---- integer routing on host (replicated bookkeeping)
    order = np.argsort(ei, kind="stable")
    tok_sorted = (np.repeat(np.arange(N_TOKENS, dtype=np.int64), TOP_K))[order]
    w_sorted = ew[order]
    counts = np.bincount(ei, minlength=N_EXPERTS)
    seg = np.concatenate(([0], np.cumsum(counts)))

    # ---- pair heavy experts with light ones; slot 0 takes the bigger one
    by_size = np.argsort(counts, kind="stable")[::-1]  # expert ids, desc count
    slot_expert = np.zeros((N_CORES, E_LOCAL), dtype=np.int64)
    for c in range(N_CORES):
        slot_expert[c, 0] = by_size[c]
        slot_expert[c, 1] = by_size[N_EXPERTS - 1 - c]
    L0 = int(max(256, -(-int(counts[slot_expert[:, 0]].max()) // 4) * 4))
    L1 = int(max(256, -(-int(counts[slot_expert[:, 1]].max()) // 4) * 4))
    Ls = (L0, L1)
    plans = [_slice_plan(L) for L in Ls]

    xb = _to_bf16(x)
    import ml_dtypes

    # ---- shard per core
    WTOT = 128 * DK * FK
    in_maps = []
    for c in range(N_CORES):
        m = {}
        w12_c = np.empty((E_LOCAL, 128, 2 * WTOT), dtype=ml_dtypes.bfloat16)
        w3_c = np.empty((E_LOCAL, 128, WTOT), dtype=ml_dtypes.bfloat16)
        for e in range(E_LOCAL):
            g = int(slot_expert[c, e])
            L = Ls[e]
            toks = tok_sorted[seg[g]:seg[g + 1]]
            wts = w_sorted[seg[g]:seg[g + 1]]
            # [128, 2, DK, L]: xa[p, 0, dk, t] = x[tok_t, dk*128 + p],
            # plane 1 pre-scaled by the routing weight
            xa = np.zeros((128, 2, DK, L), dtype=ml_dtypes.bfloat16)
            xa[:, 0, :, :len(toks)] = (
                xb[toks].T.reshape(DK, 128, -1).transpose(1, 0, 2)
            )
            xa[:, 1, :, :len(toks)] = (
                _to_bf16(x[toks] * wts[:, None]).T.reshape(DK, 128, -1).transpose(1, 0, 2)
            )
            for k, (s0, w) in enumerate(plans[e]):
                m[f"xa{e}s{k}"] = np.ascontiguousarray(xa[:, :, :, s0:s0 + w])
            w12_c[e] = _pack_w12(np.asarray(w1)[g], np.asarray(w2)[g])
            w3_c[e] = _pack_w3(np.asarray(w3)[g])
        m["w12t"] = w12_c
        m["w3t"] = w3_c
        in_maps.append(m)

    nc = _get_nc(L0, L1)
    meta = {"seg": seg, "tok_sorted": tok_sorted, "Ls": Ls, "slot_expert": slot_expert}
    return nc, in_maps, meta


def combine(results, meta):
    """Unshard per-core expert outputs and sum the top-2 contributions."""
    seg, tok_sorted, Ls = meta["seg"], meta["tok_sorted"], meta["Ls"]
    slot_expert = meta["slot_expert"]
    assign_rows = np.empty((N_TOKENS * TOP_K, D_MODEL), dtype=np.float32)
    for c in range(N_CORES):
        for e in range(E_LOCAL):
            g = int(slot_expert[c, e])
            cnt = seg[g + 1] - seg[g]
            # yt{e}s{k} is [128, DK, w] with yt[p, dk, t] = y[d=dk*128+p][t]
            parts = [
                np.asarray(results[c][f"yt{e}s{k}"], dtype=np.float32)
                .transpose(1, 0, 2)
                .reshape(D_MODEL, -1)
                for k in range(len(_slice_plan(Ls[e])))
            ]
            ytc = np.concatenate(parts, axis=1)
            assign_rows[seg[g]:seg[g + 1]] = ytc[:, :cnt].T

    by_token = np.argsort(tok_sorted, kind="stable")
    out = assign_rows[by_token].reshape(N_TOKENS, TOP_K, D_MODEL).sum(axis=1)
    return out.astype(np.float32)


def kernel(x, expert_indices, expert_weights, w1, w2, w3, _run_opts=None):
    from concourse.bass_utils import run_bass_kernel_spmd

    nc, in_maps, meta = prepare(x, expert_indices, expert_weights, w1, w2, w3)
    opts = dict(_run_opts or {})
    res = run_bass_kernel_spmd(nc, in_maps, list(range(N_CORES)), **opts)
    if _run_opts is not None:
        _run_opts["result"] = res
    return combine(res.results, meta)
